# revision 1
# baseline (speedup 1.0000x reference)
"""Sparse-attention (sliding window 512 + front 256) Trainium2 kernel.

Head-sharded across 8 NeuronCores: core c computes q-heads {2c, 2c+1} and
kv-head c//2, producing a partial output y_c = attn_out_c @ wo_c; the host
sums the 8 partials.

Layout choices:
  - q/k projections are computed transposed (qT: [dqk, seq]) directly from a
    host-provided x^T, so score matmuls need no transposes.
  - RoPE uses a "paired" head-dim basis (reals in dims 0..63, imags 64..127),
    obtained by permuting wq/wk columns on the host. Dot products are
    permutation-invariant, so scores are unchanged.
  - Sparse mask: per 128x128 tile the mask is full, causal (b<=a) or
    anti-causal (b>a); only 2 nontrivial patterns, passed as constants.
  - Softmax without max-subtraction (scores ~ N(0,1) after 1/sqrt(128) scale,
    so exp() is safe in fp32), sum fused into the exp via accum_out.
"""

import math
import sys

import numpy as np

sys.path.insert(0, "/opt/trn_rl_repo")

import concourse.bass as bass
from concourse import bacc
import concourse.mybir as mybir
import concourse.tile as tile
from concourse.bass_utils import run_bass_kernel_spmd

# Problem constants (hardcoded per contract)
S = 4096
D = 2048
NH = 16
NKV = 4
DQK = 128
DV = 128
WIN = 512
FRONT = 256
THETA = 10000.0
P = 128
NQT = S // P  # 32 query tiles
NC_ = 8  # cores
SC = 512  # seq chunk for projections
NSC = S // SC  # 8
KO = D // P  # 16 contraction chunks

F32 = mybir.dt.float32
BF16 = mybir.dt.bfloat16

NEG = -1.0e9


def _key_tiles(qt):
    """Key tiles for query tile qt: list of (kt, mask) with mask in
    {'full','causal','anti'}; tiles are contiguous groups for matmul."""
    if qt <= 5:
        tiles = [(kt, "full") for kt in range(qt)] + [(qt, "causal")]
    else:
        tiles = [(0, "full"), (1, "full"), (qt - 4, "anti")]
        tiles += [(kt, "full") for kt in range(qt - 3, qt)]
        tiles += [(qt, "causal")]
    return tiles


def build_program():
    nc = bacc.Bacc(None, target_bir_lowering=False)

    xt = nc.dram_tensor("xt", [D, S], BF16, kind="ExternalInput")
    wq_d = nc.dram_tensor("wq", [D, 2 * DQK], BF16, kind="ExternalInput")
    wk_d = nc.dram_tensor("wk", [D, DQK], BF16, kind="ExternalInput")
    wv_d = nc.dram_tensor("wv", [D, DV], BF16, kind="ExternalInput")
    wo_d = nc.dram_tensor("wo", [2 * DV, D], BF16, kind="ExternalInput")
    cos_d = nc.dram_tensor("cosd", [P, S], F32, kind="ExternalInput")
    sin_d = nc.dram_tensor("sind", [P, S], F32, kind="ExternalInput")
    mask_c_d = nc.dram_tensor("maskc", [P, P], F32, kind="ExternalInput")
    mask_a_d = nc.dram_tensor("maska", [P, P], F32, kind="ExternalInput")
    ident_d = nc.dram_tensor("ident", [P, P], BF16, kind="ExternalInput")
    y_d = nc.dram_tensor("y", [S, D], F32, kind="ExternalOutput")

    inv_sqrt_dqk = 1.0 / math.sqrt(DQK)

    with tile.TileContext(nc) as tc:
        with (
            tc.tile_pool(name="persist", bufs=1) as persist,
            tc.tile_pool(name="xchunk", bufs=2) as xpool,
            tc.tile_pool(name="stage", bufs=3) as stage,
            tc.tile_pool(name="ppool", bufs=3) as ppool,
            tc.tile_pool(name="lpool", bufs=4) as lpool,
            tc.tile_pool(name="ptpool", bufs=3) as ptpool,
            tc.tile_pool(name="ystage", bufs=2) as ypool,
            tc.tile_pool(name="psA", bufs=2, space="PSUM") as psA,
            tc.tile_pool(name="psS", bufs=2, space="PSUM") as psS,
            tc.tile_pool(name="psT", bufs=1, space="PSUM") as psT,
            tc.tile_pool(name="psO", bufs=1, space="PSUM") as psO,
        ):
            # ---- persistent SBUF tensors ----
            qT = persist.tile([P, 2, S], BF16, tag="qT")
            kT = persist.tile([P, S], BF16, tag="kT")
            v_sb = persist.tile([P, NQT, DV], BF16, tag="v")
            outT = persist.tile([P, 2, NQT, P], BF16, tag="outT")
            cos_sb = persist.tile([P, S], F32, tag="cos")
            sin_sb = persist.tile([P, S], F32, tag="sin")
            wq_sb = persist.tile([P, KO, 2 * DQK], BF16, tag="wq")
            wk_sb = persist.tile([P, KO, DQK], BF16, tag="wk")
            wv_sb = persist.tile([P, KO, DV], BF16, tag="wv")
            wo_sb = persist.tile([P, 2, D], BF16, tag="wo")
            mask_c = persist.tile([P, P], F32, tag="mc")
            mask_a = persist.tile([P, P], F32, tag="ma")
            ident = persist.tile([P, P], BF16, tag="id")

            nc.sync.dma_start(cos_sb[:], cos_d[:])
            nc.sync.dma_start(sin_sb[:], sin_d[:])
            nc.sync.dma_start(wq_sb[:], wq_d.rearrange("(ko p) m -> p ko m", p=P))
            nc.sync.dma_start(wk_sb[:], wk_d.rearrange("(ko p) m -> p ko m", p=P))
            nc.sync.dma_start(wv_sb[:], wv_d.rearrange("(ko p) m -> p ko m", p=P))
            nc.sync.dma_start(wo_sb[:], wo_d.rearrange("(h p) n -> p h n", p=P))
            nc.sync.dma_start(mask_c[:], mask_c_d[:])
            nc.sync.dma_start(mask_a[:], mask_a_d[:])
            nc.sync.dma_start(ident[:], ident_d[:])

            xt_r = xt.rearrange("(ko p) s -> p ko s", p=P)

            # ---- Phase A: projections + RoPE ----
            for sc in range(NSC):
                ssl = slice(sc * SC, (sc + 1) * SC)
                xch = xpool.tile([P, KO, SC], BF16, tag="xch")
                nc.sync.dma_start(xch[:], xt_r[:, :, ssl])

                # qT (2 head tiles) and kT, with RoPE
                for m in range(3):
                    ps = psA.tile([P, SC], F32, tag="psA")
                    if m < 2:
                        w_ap = wq_sb[:, :, m * P : (m + 1) * P]
                        dst = qT[:, m, ssl]
                    else:
                        w_ap = wk_sb
                        dst = kT[:, ssl]
                    for ko in range(KO):
                        nc.tensor.matmul(
                            ps[:],
                            w_ap[:, ko, :],
                            xch[:, ko, :],
                            start=(ko == 0),
                            stop=(ko == KO - 1),
                        )
                    st = stage.tile([P, SC], F32, tag="ropestage")
                    nc.scalar.copy(st[:], ps[:])
                    # RoPE (paired layout): rows 0:64 real, 64:128 imag
                    trc = stage.tile([P, SC], F32, tag="trc")
                    trs = stage.tile([P, SC], F32, tag="trs")
                    nc.vector.tensor_tensor(
                        trc[:], st[:], cos_sb[:, ssl], op=mybir.AluOpType.mult
                    )
                    nc.vector.tensor_tensor(
                        trs[:], st[:], sin_sb[:, ssl], op=mybir.AluOpType.mult
                    )
                    # out_r = r*c - i*s ; out_i = r*s + i*c
                    # (DVE needs equal base partitions for both SBUF inputs,
                    #  so stage the upper halves at partition 0 first)
                    his = stage.tile([64, SC], F32, tag="his")
                    hic = stage.tile([64, SC], F32, tag="hic")
                    nc.scalar.copy(his[:], trs[64:128])
                    nc.scalar.copy(hic[:], trc[64:128])
                    nc.vector.tensor_tensor(
                        dst[0:64], trc[0:64], his[:], op=mybir.AluOpType.subtract
                    )
                    nc.vector.tensor_tensor(
                        dst[64:128], trs[0:64], hic[:], op=mybir.AluOpType.add
                    )

                # v natural layout: [seq, dv] per key tile
                for j in range(SC // P):
                    kt_idx = sc * (SC // P) + j
                    psv = psA.tile([P, SC], F32, tag="psA")
                    for ko in range(KO):
                        nc.tensor.matmul(
                            psv[:, :DV],
                            xch[:, ko, j * P : (j + 1) * P],
                            wv_sb[:, ko, :],
                            start=(ko == 0),
                            stop=(ko == KO - 1),
                        )
                    nc.scalar.copy(v_sb[:, kt_idx, :], psv[:, :DV])

            # ---- Phase B: attention ----
            for h in range(2):
                for qt in range(NQT):
                    tiles = _key_tiles(qt)
                    nk = len(tiles)
                    w = nk * P
                    qh = qT[:, h, qt * P : (qt + 1) * P]

                    ps_s = psS.tile([P, 7 * P], F32, tag="psS", name="ps_s")
                    ps_s = ps_s[:, :w]
                    # contiguous matmul groups, split at psum bank (512) bounds
                    groups = []  # (dst_lo, kt_lo, width)
                    pos = 0
                    i = 0
                    while i < nk:
                        j = i
                        while j + 1 < nk and tiles[j + 1][0] == tiles[j][0] + 1:
                            j += 1
                        glo, gw = tiles[i][0], (j - i + 1) * P
                        # split so no matmul crosses a 512-col psum boundary
                        off = 0
                        while off < gw:
                            room = 512 - ((pos + off) % 512)
                            take = min(gw - off, room, 512)
                            groups.append((pos + off, glo * P + off, take))
                            off += take
                        pos += gw
                        i = j + 1
                    for dst_lo, src_lo, gw in groups:
                        nc.tensor.matmul(
                            ps_s[:, dst_lo : dst_lo + gw],
                            qh,
                            kT[:, src_lo : src_lo + gw],
                            start=True,
                            stop=True,
                        )
                    # masks
                    for idx, (kt, mk) in enumerate(tiles):
                        if mk == "full":
                            continue
                        msk = mask_c if mk == "causal" else mask_a
                        nc.vector.tensor_tensor(
                            ps_s[:, idx * P : (idx + 1) * P],
                            ps_s[:, idx * P : (idx + 1) * P],
                            msk[:],
                            op=mybir.AluOpType.add,
                        )
                    # exp + row-sum
                    p_sb = ppool.tile([P, 7 * P], BF16, tag="p", name="p_sb")
                    p_sb = p_sb[:, :w]
                    lsum = lpool.tile([P, 1], F32, tag="l")
                    nc.scalar.activation(
                        p_sb,
                        ps_s,
                        mybir.ActivationFunctionType.Exp,
                        scale=inv_sqrt_dqk,
                        accum_out=lsum[:],
                    )
                    rl = lpool.tile([P, 1], F32, tag="rl")
                    nc.vector.reciprocal(rl[:], lsum[:])
                    nc.vector.tensor_tensor(
                        p_sb, p_sb, rl.to_broadcast((P, w)), op=mybir.AluOpType.mult
                    )
                    # transpose p tiles; accumulate out
                    ps_o = psO.tile([P, P], F32, tag="psO")
                    for idx, (kt, mk) in enumerate(tiles):
                        ps_t = psT.tile([P, P], BF16, tag="psT")
                        nc.tensor.transpose(
                            ps_t[:], p_sb[:, idx * P : (idx + 1) * P], ident[:]
                        )
                        pt_sb = ptpool.tile([P, P], BF16, tag="pt")
                        nc.scalar.copy(pt_sb[:], ps_t[:])
                        nc.tensor.matmul(
                            ps_o[:],
                            v_sb[:, kt, :],
                            pt_sb[:],
                            start=(idx == 0),
                            stop=(idx == nk - 1),
                        )
                    nc.scalar.copy(outT[:, h, qt, :], ps_o[:])

            # ---- Phase C: y = outT.T @ wo ----
            for st_i in range(NQT):
                ys = ypool.tile([P, D], F32, tag="y")
                for nn in range(4):
                    ps_y = psA.tile([P, SC], F32, tag="psA")
                    for h in range(2):
                        nc.tensor.matmul(
                            ps_y[:],
                            outT[:, h, st_i, :],
                            wo_sb[:, h, nn * SC : (nn + 1) * SC],
                            start=(h == 0),
                            stop=(h == 1),
                        )
                    nc.scalar.copy(ys[:, nn * SC : (nn + 1) * SC], ps_y[:])
                nc.sync.dma_start(y_d[st_i * P : (st_i + 1) * P, :], ys[:])

    return nc


_PROGRAM = None


def _get_program():
    global _PROGRAM
    if _PROGRAM is None:
        _PROGRAM = build_program()
        # Bacc legalization (register alloc, 1-wait-per-instruction split)
        # must run before serialization; the prebuilt-nc PJRT path does not
        # call finalize itself.
        _PROGRAM.finalize()
    return _PROGRAM


def _host_inputs(x, wq, wk, wv, wo):
    """Build per-core input maps (host-side sharding + preprocessing)."""
    x2 = np.asarray(x).reshape(S, D).astype(np.float32)
    xt_bf = x2.T.astype(ml_bf16)

    # paired RoPE basis permutation within each head
    perm = np.concatenate([np.arange(0, DQK, 2), np.arange(1, DQK, 2)])
    wq_p = np.asarray(wq).reshape(D, NH, DQK)[:, :, perm]
    wk_p = np.asarray(wk).reshape(D, NKV, DQK)[:, :, perm]
    wv_r = np.asarray(wv).reshape(D, NKV, DV)
    wo_r = np.asarray(wo).reshape(NH, DV, D)

    inv_freq = 1.0 / (THETA ** (np.arange(0, DQK, 2)[: DQK // 2] / DQK))
    t = np.arange(S, dtype=np.float64)
    ang = np.outer(t, inv_freq)  # (S, 64)
    cos_half = np.cos(ang).T.astype(np.float32)  # (64, S)
    sin_half = np.sin(ang).T.astype(np.float32)
    cos_dup = np.concatenate([cos_half, cos_half], 0)  # (128, S)
    sin_dup = np.concatenate([sin_half, sin_half], 0)

    a = np.arange(P)[:, None]
    b = np.arange(P)[None, :]
    mask_c = np.where(b <= a, 0.0, NEG).astype(np.float32)
    mask_a = np.where(b > a, 0.0, NEG).astype(np.float32)
    ident = np.eye(P, dtype=np.float32).astype(ml_bf16)

    in_maps = []
    for c in range(NC_):
        kvh = c // 2
        in_maps.append(
            {
                "xt": xt_bf,
                "wq": np.ascontiguousarray(
                    wq_p[:, 2 * c : 2 * c + 2, :].reshape(D, 2 * DQK)
                ).astype(ml_bf16),
                "wk": np.ascontiguousarray(wk_p[:, kvh, :]).astype(ml_bf16),
                "wv": np.ascontiguousarray(wv_r[:, kvh, :]).astype(ml_bf16),
                "wo": np.ascontiguousarray(
                    wo_r[2 * c : 2 * c + 2].reshape(2 * DV, D)
                ).astype(ml_bf16),
                "cosd": cos_dup,
                "sind": sin_dup,
                "maskc": mask_c,
                "maska": mask_a,
                "ident": ident,
            }
        )
    return in_maps


try:
    import ml_dtypes

    ml_bf16 = ml_dtypes.bfloat16
except ImportError:  # pragma: no cover
    ml_bf16 = np.float32


def kernel(x, wq, wk, wv, wo, _trace=False, _trace_kwargs=None):
    nc = _get_program()
    in_maps = _host_inputs(x, wq, wk, wv, wo)
    res = run_bass_kernel_spmd(
        nc, in_maps, list(range(NC_)), trace=_trace, **(_trace_kwargs or {})
    )
    y = np.zeros((S, D), np.float32)
    for r in res.results:
        y += np.asarray(r["y"], np.float32)
    out = y.reshape(1, S, D)
    if _trace:
        return out, res
    return out



# revision 5
# speedup vs baseline: 1.2780x; 1.2780x over previous
"""Sparse-attention (sliding window 512 + front 256) Trainium2 kernel, v2.

Sequence-sharded across 8 NeuronCores: core c owns queries [512c, 512c+512)
and computes ALL 16 heads for them, including the full output projection, so
per-core output is a disjoint y slice [512, 2048] (no cross-core reduction).

Key layout trick: scores are computed TRANSPOSED (sT[k, q] = kT_tile^T @ qT),
so the exp() output is already the [keys, q] operand the attn@V matmul needs
as rhs — no PE transposes and no PSUM->SBUF staging copies (which dominated
the v1 profile: 383us of scalar-engine ACTIVATE copies).

Uniform single program across cores; all per-core variation (which key tiles
are front/band/causal/padding) is data: a multiplicative {0,1} mask tensor B
applied to exp(scores). Padding key tiles have x=0 so their scores are exactly
0 -> exp=1 -> masked to 0; no -inf anywhere.

Softmax row-sum (over keys = PSUM partition dim) is a ones-vector matmul on
the PE; the reciprocal is transposed back to [128,1] with a tiny PE transpose
and applied to the attn@V output with a free-dim broadcast multiply.

Per-core packed key layout (NT=10 tiles of 128):
  cols 0:256    front tiles (positions 0:256)
  cols 256:1280 band tiles (positions qlo-512 .. qlo+512, zero-padded if <0)
Query tile qtl (0..3) attends key-list positions:
  [front0, front1, band qtl, band qtl+1, ..., band qtl+4]   (7 tiles)
Front tiles use only the front_ok mask condition, band tiles only the band
condition (the two are disjoint: j<=i-512 vs j>i-512), so the duplicated
front/band tiles on cores 0-1 never double count.
"""

import math
import sys

import numpy as np

sys.path.insert(0, "/opt/trn_rl_repo")

import concourse.bass as bass
from concourse import bacc
import concourse.mybir as mybir
import concourse.tile as tile
from concourse.bass_utils import run_bass_kernel_spmd

# Problem constants (hardcoded per contract)
S = 4096
D = 2048
NH = 16
NKV = 4
NREP = NH // NKV
DQK = 128
DV = 128
WIN = 512
FRONT = 256
THETA = 10000.0
P = 128
NC_ = 8          # cores
SC = S // NC_    # 512 queries per core
KO = D // P      # 16 contraction chunks
NT = 10          # packed key tiles per core (2 front + 8 band)
NKC = NT * P     # 1280 packed key positions
NQTL = 4         # query tiles per core
NKT = 7          # key tiles per query tile

F32 = mybir.dt.float32
BF16 = mybir.dt.bfloat16

try:
    import ml_dtypes

    ml_bf16 = ml_dtypes.bfloat16
except ImportError:  # pragma: no cover
    ml_bf16 = np.float32


def build_program():
    nc = bacc.Bacc(None, target_bir_lowering=False)

    x_d = nc.dram_tensor("xp", [P, KO, NKC], BF16, kind="ExternalInput")
    wq_d = nc.dram_tensor("wq", [P, 4, KO, 4 * DQK], BF16, kind="ExternalInput")
    wk_d = nc.dram_tensor("wk", [P, KO, NKV * DQK], BF16, kind="ExternalInput")
    wv_d = nc.dram_tensor("wv", [P, KO, NKV * DV], BF16, kind="ExternalInput")
    wo_d = nc.dram_tensor("wo", [P, 4, NH, SC], BF16, kind="ExternalInput")
    cos_d = nc.dram_tensor("cosd", [P, NKC], F32, kind="ExternalInput")
    sin_d = nc.dram_tensor("sind", [P, NKC], F32, kind="ExternalInput")
    b_d = nc.dram_tensor("bmask", [P, NQTL * NKT * P], BF16, kind="ExternalInput")
    y_d = nc.dram_tensor("y", [SC, D], F32, kind="ExternalOutput")

    inv_sqrt_dqk = 1.0 / math.sqrt(DQK)
    qc0 = NKC - SC  # first packed col of this core's own queries (768)

    with tile.TileContext(nc) as tc:
        with (
            tc.tile_pool(name="persist", bufs=1) as persist,
            tc.tile_pool(name="ps", bufs=2, space="PSUM") as ps,
            tc.tile_pool(name="psO", bufs=2, space="PSUM") as psO,
            tc.tile_pool(name="psL", bufs=1, space="PSUM") as psL,
            tc.tile_pool(name="psRT", bufs=1, space="PSUM") as psRT,
        ):
            # ---- persistent SBUF (lives through both phases) ----
            qT = persist.tile([P, NH, SC], BF16, tag="qT")
            kT = persist.tile([P, NKV, NKC], BF16, tag="kT")
            v_sb = persist.tile([P, NT, NKV * DV], BF16, tag="v")
            outT = persist.tile([P, NH, SC], BF16, tag="outT")
            b_sb = persist.tile([P, NQTL * NKT * P], BF16, tag="bm")
            ones_sb = persist.tile([P, 1], BF16, tag="ones")
            ones_row = persist.tile([1, P], F32, tag="onesr")

            nc.sync.dma_start(b_sb[:], b_d[:])
            nc.vector.memset(ones_sb[:], 1.0)
            nc.vector.memset(ones_row[:], 1.0)

            def rope(dst, psrc, cosap, sinap, pool, w):
                """dst(bf16) = RoPE(psrc) in the paired [re(64); im(64)] basis.

                sw = [-im; re]; dst = psrc*cos + sw*sin.
                """
                sw = pool.tile([P, w], F32, tag="sw")
                nc.scalar.mul(sw[0:64], psrc[64:128], -1.0)
                nc.scalar.copy(sw[64:128], psrc[0:64])
                trc = pool.tile([P, w], F32, tag="trc")
                nc.vector.tensor_tensor(
                    trc[:], psrc, cosap, op=mybir.AluOpType.mult
                )
                nc.vector.tensor_tensor(
                    sw[:], sw[:], sinap, op=mybir.AluOpType.mult
                )
                nc.vector.tensor_tensor(
                    dst, trc[:], sw[:], op=mybir.AluOpType.add
                )

            # ---- Phase A: projections + RoPE (x/weights pool freed after) ----
            with tc.tile_pool(name="phA", bufs=1) as pa, tc.tile_pool(
                name="wqs", bufs=2
            ) as wqs, tc.tile_pool(name="ropep", bufs=2) as rp:
                x_sb = pa.tile([P, KO, NKC], BF16, tag="x")
                cos_sb = pa.tile([P, NKC], F32, tag="cos")
                sin_sb = pa.tile([P, NKC], F32, tag="sin")
                wk_sb = pa.tile([P, KO, NKV * DQK], BF16, tag="wk")
                wv_sb = pa.tile([P, KO, NKV * DV], BF16, tag="wv")

                nc.sync.dma_start(x_sb[:], x_d[:])
                nc.sync.dma_start(cos_sb[:], cos_d[:])
                nc.sync.dma_start(sin_sb[:], sin_d[:])
                nc.sync.dma_start(wk_sb[:], wk_d[:])
                nc.sync.dma_start(wv_sb[:], wv_d[:])

                # k projection + RoPE: kT[dqk, keys] per kv head
                for kvh in range(NKV):
                    for c0 in range(0, NKC, 512):
                        cw = min(512, NKC - c0)
                        psk = ps.tile([P, 7 * P], F32, tag="big", name="psk")
                        psk = psk[:, :cw]
                        for ko in range(KO):
                            nc.tensor.matmul(
                                psk,
                                wk_sb[:, ko, kvh * DQK : (kvh + 1) * DQK],
                                x_sb[:, ko, c0 : c0 + cw],
                                start=(ko == 0),
                                stop=(ko == KO - 1),
                            )
                        rope(
                            kT[:, kvh, c0 : c0 + cw],
                            psk,
                            cos_sb[:, c0 : c0 + cw],
                            sin_sb[:, c0 : c0 + cw],
                            rp,
                            cw,
                        )

                # v projection (natural [keys, dv], all 4 kv heads at once)
                for t in range(NT):
                    psv = ps.tile([P, 7 * P], F32, tag="big", name="psv")
                    psv = psv[:, : NKV * DV]
                    for ko in range(KO):
                        nc.tensor.matmul(
                            psv,
                            x_sb[:, ko, t * P : (t + 1) * P],
                            wv_sb[:, ko, :],
                            start=(ko == 0),
                            stop=(ko == KO - 1),
                        )
                    nc.vector.tensor_copy(v_sb[:, t, :], psv)

                # q projection + RoPE, streaming wq in 4-head groups
                for g in range(4):
                    wq_g = wqs.tile([P, KO, 4 * DQK], BF16, tag="wqg")
                    nc.sync.dma_start(wq_g[:], wq_d[:, g])
                    for hh in range(4):
                        h = 4 * g + hh
                        psq = ps.tile([P, 7 * P], F32, tag="big", name="psq")
                        psq = psq[:, :SC]
                        for ko in range(KO):
                            nc.tensor.matmul(
                                psq,
                                wq_g[:, ko, hh * DQK : (hh + 1) * DQK],
                                x_sb[:, ko, qc0:NKC],
                                start=(ko == 0),
                                stop=(ko == KO - 1),
                            )
                        rope(
                            qT[:, h, :],
                            psq,
                            cos_sb[:, qc0:NKC],
                            sin_sb[:, qc0:NKC],
                            rp,
                            SC,
                        )

            # ---- Phase B: attention (transposed scores) ----
            with tc.tile_pool(name="phB", bufs=3) as pb, tc.tile_pool(
                name="phBs", bufs=2
            ) as pbs:
                for h in range(NH):
                    kvh = h // NREP
                    for qtl in range(NQTL):
                        # 7 key tiles: packed cols [0, 128, 256+128*qtl ...]
                        cols = [0, P] + [
                            (2 + qtl + j) * P for j in range(5)
                        ]
                        qsl = qT[:, h, qtl * P : (qtl + 1) * P]
                        pss = ps.tile([P, NKT * P], F32, tag="big", name="pss")
                        for i, c0 in enumerate(cols):
                            nc.tensor.matmul(
                                pss[:, i * P : (i + 1) * P],
                                kT[:, kvh, c0 : c0 + P],
                                qsl,
                                start=True,
                                stop=True,
                            )
                        pT = pb.tile([P, NKT * P], BF16, tag="pT")
                        nc.scalar.activation(
                            pT[:],
                            pss[:],
                            mybir.ActivationFunctionType.Exp,
                            scale=inv_sqrt_dqk,
                        )
                        bsl = b_sb[
                            :, qtl * NKT * P : (qtl + 1) * NKT * P
                        ]
                        nc.vector.tensor_tensor(
                            pT[:], pT[:], bsl, op=mybir.AluOpType.mult
                        )
                        # lsum over keys (partition dim) via ones-matmul
                        psl = psL.tile([1, P], F32, tag="l")
                        for i in range(NKT):
                            nc.tensor.matmul(
                                psl[:],
                                ones_sb[:],
                                pT[:, i * P : (i + 1) * P],
                                start=(i == 0),
                                stop=(i == NKT - 1),
                            )
                        pso = psO.tile([P, P], F32, tag="o")
                        for i, c0 in enumerate(cols):
                            nc.tensor.matmul(
                                pso[:],
                                v_sb[:, c0 // P, kvh * DV : (kvh + 1) * DV],
                                pT[:, i * P : (i + 1) * P],
                                start=(i == 0),
                                stop=(i == NKT - 1),
                            )
                        # 1/lsum, broadcast to all partitions via rank-1 matmul
                        irl = pbs.tile([1, P], F32, tag="irl")
                        nc.vector.reciprocal(irl[:], psl[:])
                        psbc = psRT.tile([P, P], F32, tag="bc")
                        nc.tensor.matmul(
                            psbc[:], ones_row[:], irl[:], start=True, stop=True
                        )
                        rlbc = pbs.tile([P, P], BF16, tag="rlbc")
                        nc.scalar.copy(rlbc[:], psbc[:])
                        nc.vector.tensor_tensor(
                            outT[:, h, qtl * P : (qtl + 1) * P],
                            pso[:],
                            rlbc[:],
                            op=mybir.AluOpType.mult,
                        )

            # ---- Phase C: y = outT^T @ wo (stream wo in n-chunks) ----
            with tc.tile_pool(name="phC", bufs=2) as pc, tc.tile_pool(
                name="phCy", bufs=4
            ) as pcy:
                y_tiles = [
                    pcy.tile([P, D], F32, tag="y", name=f"y{i}")
                    for i in range(NQTL)
                ]
                for ncl in range(4):
                    wo_g = pc.tile([P, NH, SC], BF16, tag="wog")
                    nc.sync.dma_start(wo_g[:], wo_d[:, ncl])
                    for qtl in range(NQTL):
                        psy = ps.tile([P, 7 * P], F32, tag="big", name="psy")
                        psy = psy[:, :SC]
                        for h in range(NH):
                            nc.tensor.matmul(
                                psy,
                                outT[:, h, qtl * P : (qtl + 1) * P],
                                wo_g[:, h, :],
                                start=(h == 0),
                                stop=(h == NH - 1),
                            )
                        nc.vector.tensor_copy(
                            y_tiles[qtl][:, ncl * SC : (ncl + 1) * SC], psy
                        )
                for qtl in range(NQTL):
                    nc.sync.dma_start(
                        y_d[qtl * P : (qtl + 1) * P, :], y_tiles[qtl][:]
                    )

    return nc


_PROGRAM = None


def _get_program():
    global _PROGRAM
    if _PROGRAM is None:
        _PROGRAM = build_program()
        _PROGRAM.finalize()
    return _PROGRAM


def _host_inputs(x, wq, wk, wv, wo):
    """Per-core input packing (all arrays contiguous, uniform shapes)."""
    x2 = np.asarray(x, np.float32).reshape(S, D)
    xT = np.ascontiguousarray(x2.T)  # [D, S]
    xr = xT.reshape(KO, P, S)  # [ko, p, s]

    # paired RoPE basis permutation within each head
    perm = np.concatenate([np.arange(0, DQK, 2), np.arange(1, DQK, 2)])
    wq_p = np.asarray(wq, np.float32).reshape(D, NH, DQK)[:, :, perm]
    wk_p = np.asarray(wk, np.float32).reshape(D, NKV, DQK)[:, :, perm]
    wv_r = np.asarray(wv, np.float32).reshape(D, NKV * DV)
    wo_r = np.asarray(wo, np.float32).reshape(NH, DV, D)

    # device layouts independent of core
    wq_dev = np.ascontiguousarray(
        wq_p.reshape(KO, P, NH, DQK)  # [ko, p, h, dqk]
        .reshape(KO, P, 4, 4 * DQK)  # group 4 heads
        .transpose(1, 2, 0, 3)  # [p, g, ko, 4*dqk]
    ).astype(ml_bf16)
    wk_dev = np.ascontiguousarray(
        wk_p.reshape(KO, P, NKV * DQK).transpose(1, 0, 2)
    ).astype(ml_bf16)
    wv_dev = np.ascontiguousarray(
        wv_r.reshape(KO, P, NKV * DV).transpose(1, 0, 2)
    ).astype(ml_bf16)
    wo_dev = np.ascontiguousarray(
        wo_r.reshape(NH, DV, 4, SC).transpose(1, 2, 0, 3)  # [dv, ncl, h, sc]
    ).astype(ml_bf16)

    inv_freq = 1.0 / (THETA ** (np.arange(0, DQK, 2)[: DQK // 2] / DQK))

    in_maps = []
    for c in range(NC_):
        qlo = c * SC
        band_lo = qlo - WIN
        # packed key positions; garbage (pos<0) -> position 0, zero x
        pos = np.empty(NKC, np.int64)
        pos[: FRONT] = np.arange(FRONT)
        pos[FRONT:] = band_lo + np.arange(NKC - FRONT)
        valid = pos >= 0
        pos_c = np.where(valid, pos, 0)

        xp = xr[:, :, pos_c] * valid[None, None, :]  # [ko, p, nkc]
        xp = np.ascontiguousarray(xp.transpose(1, 0, 2)).astype(ml_bf16)

        ang = np.outer(pos_c.astype(np.float64), inv_freq)  # (nkc, 64)
        cos_h = np.cos(ang).T.astype(np.float32)  # (64, nkc)
        sin_h = np.sin(ang).T.astype(np.float32)
        cos_p = np.ascontiguousarray(np.concatenate([cos_h, cos_h], 0))
        sin_p = np.ascontiguousarray(np.concatenate([sin_h, sin_h], 0))

        # B mask [r, qtl, i, cq] -> [P, NQTL*NKT*P]
        r = np.arange(P)
        cq = np.arange(P)
        B = np.zeros((P, NQTL, NKT, P), np.float32)
        for qtl in range(NQTL):
            qpos = qlo + qtl * P + cq[None, :]  # [1, P]
            for i in range(NKT):
                if i < 2:
                    kpos = i * P + r[:, None]  # [P, 1]
                    allowed = (kpos < FRONT) & (kpos <= qpos - WIN)
                else:
                    kpos = band_lo + (qtl + i - 2) * P + r[:, None]
                    allowed = (
                        (kpos >= 0) & (kpos <= qpos) & (kpos > qpos - WIN)
                    )
                B[:, qtl, i, :] = allowed
        Bp = np.ascontiguousarray(B.reshape(P, NQTL * NKT * P)).astype(
            ml_bf16
        )

        in_maps.append(
            {
                "xp": xp,
                "wq": wq_dev,
                "wk": wk_dev,
                "wv": wv_dev,
                "wo": wo_dev,
                "cosd": cos_p,
                "sind": sin_p,
                "bmask": Bp,
            }
        )
    return in_maps


def kernel(x, wq, wk, wv, wo, _trace=False, _trace_kwargs=None):
    nc = _get_program()
    in_maps = _host_inputs(x, wq, wk, wv, wo)
    res = run_bass_kernel_spmd(
        nc, in_maps, list(range(NC_)), trace=_trace, **(_trace_kwargs or {})
    )
    y = np.concatenate(
        [np.asarray(r["y"], np.float32) for r in res.results], axis=0
    )
    out = y.reshape(1, S, D)
    if _trace:
        return out, res
    return out


# revision 8
# speedup vs baseline: 1.4563x; 1.1396x over previous
"""Sparse-attention (sliding window 512 + front 256) Trainium2 kernel, v2.

Sequence-sharded across 8 NeuronCores: core c owns queries [512c, 512c+512)
and computes ALL 16 heads for them, including the full output projection, so
per-core output is a disjoint y slice [512, 2048] (no cross-core reduction).

Key layout trick: scores are computed TRANSPOSED (sT[k, q] = kT_tile^T @ qT),
so the exp() output is already the [keys, q] operand the attn@V matmul needs
as rhs — no PE transposes and no PSUM->SBUF staging copies (which dominated
the v1 profile: 383us of scalar-engine ACTIVATE copies).

Uniform single program across cores; all per-core variation (which key tiles
are front/band/causal/padding) is data: a multiplicative {0,1} mask tensor B
applied to exp(scores). Padding key tiles have x=0 so their scores are exactly
0 -> exp=1 -> masked to 0; no -inf anywhere.

Softmax row-sum (over keys = PSUM partition dim) is a ones-vector matmul on
the PE; the reciprocal is transposed back to [128,1] with a tiny PE transpose
and applied to the attn@V output with a free-dim broadcast multiply.

Per-core packed key layout (NT=10 tiles of 128):
  cols 0:256    front tiles (positions 0:256)
  cols 256:1280 band tiles (positions qlo-512 .. qlo+512, zero-padded if <0)
Query tile qtl (0..3) attends key-list positions:
  [front0, front1, band qtl, band qtl+1, ..., band qtl+4]   (7 tiles)
Front tiles use only the front_ok mask condition, band tiles only the band
condition (the two are disjoint: j<=i-512 vs j>i-512), so the duplicated
front/band tiles on cores 0-1 never double count.
"""

import math
import sys

import numpy as np

sys.path.insert(0, "/opt/trn_rl_repo")

import concourse.bass as bass
from concourse import bacc
import concourse.mybir as mybir
import concourse.tile as tile
from concourse.bass_utils import run_bass_kernel_spmd

# Problem constants (hardcoded per contract)
S = 4096
D = 2048
NH = 16
NKV = 4
NREP = NH // NKV
DQK = 128
DV = 128
WIN = 512
FRONT = 256
THETA = 10000.0
P = 128
NC_ = 8          # cores
SC = S // NC_    # 512 queries per core
KO = D // P      # 16 contraction chunks
NT = 10          # packed key tiles per core (2 front + 8 band)
NKC = NT * P     # 1280 packed key positions
NQTL = 4         # query tiles per core
NKT = 7          # key tiles per query tile

F32 = mybir.dt.float32
BF16 = mybir.dt.bfloat16

try:
    import ml_dtypes

    ml_bf16 = ml_dtypes.bfloat16
except ImportError:  # pragma: no cover
    ml_bf16 = np.float32


def build_program():
    nc = bacc.Bacc(None, target_bir_lowering=False)

    x_d = nc.dram_tensor("xp", [P, KO, NKC], BF16, kind="ExternalInput")
    wq_d = nc.dram_tensor("wq", [P, 4, KO, 4 * DQK], BF16, kind="ExternalInput")
    wk_d = nc.dram_tensor("wk", [P, KO, NKV * DQK], BF16, kind="ExternalInput")
    wv_d = nc.dram_tensor("wv", [P, KO, NKV * DV], BF16, kind="ExternalInput")
    wo_d = nc.dram_tensor("wo", [P, 4, NH, SC], BF16, kind="ExternalInput")
    cos_d = nc.dram_tensor("cosd", [P, NKC], F32, kind="ExternalInput")
    sin_d = nc.dram_tensor("sind", [P, NKC], F32, kind="ExternalInput")
    b_d = nc.dram_tensor("bmask", [P, NT * SC], BF16, kind="ExternalInput")
    y_d = nc.dram_tensor("y", [SC, D], F32, kind="ExternalOutput")

    inv_sqrt_dqk = 1.0 / math.sqrt(DQK)
    qc0 = NKC - SC  # first packed col of this core's own queries (768)

    with tile.TileContext(nc) as tc:
        with (
            tc.tile_pool(name="persist", bufs=1) as persist,
            tc.tile_pool(name="ps", bufs=4, space="PSUM") as ps,
            tc.tile_pool(name="psO", bufs=2, space="PSUM") as psO,
            tc.tile_pool(name="psL", bufs=2, space="PSUM") as psL,
        ):
            # ---- persistent SBUF (lives through both phases) ----
            qT = persist.tile([P, NH, SC], BF16, tag="qT")
            kT = persist.tile([P, NKV, NKC], BF16, tag="kT")
            v_sb = persist.tile([P, NT, NKV * DV], BF16, tag="v")
            outT = persist.tile([P, NH, SC], BF16, tag="outT")
            b_sb = persist.tile([P, NT * SC], BF16, tag="bm")
            ones_sb = persist.tile([P, 1], BF16, tag="ones")
            ones_row = persist.tile([1, P], F32, tag="onesr")

            nc.sync.dma_start(b_sb[:], b_d[:])
            nc.vector.memset(ones_sb[:], 1.0)
            nc.vector.memset(ones_row[:], 1.0)

            def rope(dst, psrc, cosap, sinap, pool, w):
                """dst(bf16) = RoPE(psrc) in the paired [re(64); im(64)] basis.

                sw = [-im; re]; dst = psrc*cos + sw*sin.
                """
                sw = pool.tile([P, w], F32, tag="sw")
                nc.scalar.mul(sw[0:64], psrc[64:128], -1.0)
                nc.scalar.copy(sw[64:128], psrc[0:64])
                trc = pool.tile([P, w], F32, tag="trc")
                nc.vector.tensor_tensor(
                    trc[:], psrc, cosap, op=mybir.AluOpType.mult
                )
                nc.vector.tensor_tensor(
                    sw[:], sw[:], sinap, op=mybir.AluOpType.mult
                )
                nc.vector.tensor_tensor(
                    dst, trc[:], sw[:], op=mybir.AluOpType.add
                )

            # ---- Phase A: projections + RoPE (x/weights pool freed after) ----
            with tc.tile_pool(name="phA", bufs=1) as pa, tc.tile_pool(
                name="wqs", bufs=2
            ) as wqs, tc.tile_pool(name="ropep", bufs=2) as rp:
                x_sb = pa.tile([P, KO, NKC], BF16, tag="x")
                cos_sb = pa.tile([P, NKC], F32, tag="cos")
                sin_sb = pa.tile([P, NKC], F32, tag="sin")
                wk_sb = pa.tile([P, KO, NKV * DQK], BF16, tag="wk")
                wv_sb = pa.tile([P, KO, NKV * DV], BF16, tag="wv")

                nc.sync.dma_start(x_sb[:], x_d[:])
                nc.sync.dma_start(cos_sb[:], cos_d[:])
                nc.sync.dma_start(sin_sb[:], sin_d[:])
                nc.sync.dma_start(wk_sb[:], wk_d[:])
                nc.sync.dma_start(wv_sb[:], wv_d[:])

                # k projection + RoPE: kT[dqk, keys] per kv head
                for kvh in range(NKV):
                    for c0 in range(0, NKC, 512):
                        cw = min(512, NKC - c0)
                        psk = ps.tile([P, SC], F32, tag="big", name="psk")
                        psk = psk[:, :cw]
                        for ko in range(KO):
                            nc.tensor.matmul(
                                psk,
                                wk_sb[:, ko, kvh * DQK : (kvh + 1) * DQK],
                                x_sb[:, ko, c0 : c0 + cw],
                                start=(ko == 0),
                                stop=(ko == KO - 1),
                            )
                        rope(
                            kT[:, kvh, c0 : c0 + cw],
                            psk,
                            cos_sb[:, c0 : c0 + cw],
                            sin_sb[:, c0 : c0 + cw],
                            rp,
                            cw,
                        )

                # v projection (natural [keys, dv], all 4 kv heads at once)
                for t in range(NT):
                    psv = ps.tile([P, SC], F32, tag="big", name="psv")
                    psv = psv[:, : NKV * DV]
                    for ko in range(KO):
                        nc.tensor.matmul(
                            psv,
                            x_sb[:, ko, t * P : (t + 1) * P],
                            wv_sb[:, ko, :],
                            start=(ko == 0),
                            stop=(ko == KO - 1),
                        )
                    nc.vector.tensor_copy(v_sb[:, t, :], psv)

                # q projection + RoPE, streaming wq in 4-head groups
                for g in range(4):
                    wq_g = wqs.tile([P, KO, 4 * DQK], BF16, tag="wqg")
                    nc.sync.dma_start(wq_g[:], wq_d[:, g])
                    for hh in range(4):
                        h = 4 * g + hh
                        psq = ps.tile([P, SC], F32, tag="big", name="psq")
                        psq = psq[:, :SC]
                        for ko in range(KO):
                            nc.tensor.matmul(
                                psq,
                                wq_g[:, ko, hh * DQK : (hh + 1) * DQK],
                                x_sb[:, ko, qc0:NKC],
                                start=(ko == 0),
                                stop=(ko == KO - 1),
                            )
                        rope(
                            qT[:, h, :],
                            psq,
                            cos_sb[:, qc0:NKC],
                            sin_sb[:, qc0:NKC],
                            rp,
                            SC,
                        )

            # ---- Phase B: attention (transposed scores, per-head tile rows) --
            # Per key tile t, the q columns that can attend it:
            #   front tiles (t=0,1): all 512;  band tile b: qtl in [b-4, b].
            qr = {0: (0, SC), 1: (0, SC)}
            for b in range(8):
                lo = max(0, b - 4) * P
                hi = (min(3, b) + 1) * P
                qr[2 + b] = (lo, hi - lo)
            with tc.tile_pool(name="phB", bufs=3) as pb, tc.tile_pool(
                name="phBs", bufs=2
            ) as pbs:
                for h in range(NH):
                    kvh = h // NREP
                    pso = psO.tile([P, SC], F32, tag="o")
                    psl = psL.tile([1, SC], F32, tag="l")
                    for ti in range(NT):
                        q0, qw = qr[ti]
                        first = ti == 0
                        last = ti == NT - 1
                        pst = ps.tile([P, SC], F32, tag="big", name="pst")
                        pst = pst[:, :qw]
                        nc.tensor.matmul(
                            pst,
                            kT[:, kvh, ti * P : (ti + 1) * P],
                            qT[:, h, q0 : q0 + qw],
                            start=True,
                            stop=True,
                        )
                        pTt = pb.tile([P, SC], BF16, tag="pT", name="pTt")
                        pTt = pTt[:, :qw]
                        nc.scalar.activation(
                            pTt,
                            pst,
                            mybir.ActivationFunctionType.Exp,
                            scale=inv_sqrt_dqk,
                        )
                        nc.vector.tensor_tensor(
                            pTt,
                            pTt,
                            b_sb[:, ti * SC + q0 : ti * SC + q0 + qw],
                            op=mybir.AluOpType.mult,
                        )
                        nc.tensor.matmul(
                            psl[:, q0 : q0 + qw],
                            ones_sb[:],
                            pTt,
                            start=first,
                            stop=last,
                            skip_group_check=True,
                        )
                        nc.tensor.matmul(
                            pso[:, q0 : q0 + qw],
                            v_sb[:, ti, kvh * DV : (kvh + 1) * DV],
                            pTt,
                            start=first,
                            stop=last,
                            skip_group_check=True,
                        )
                    # normalize: 1/lsum broadcast via rank-1 ones matmul
                    ils = pbs.tile([1, SC], F32, tag="ils")
                    nc.vector.reciprocal_approx_fast(ils[:], psl[:])
                    psbc = ps.tile([P, SC], F32, tag="big", name="psbc")
                    nc.tensor.matmul(
                        psbc[:], ones_row[:], ils[:], start=True, stop=True
                    )
                    rlbc = pbs.tile([P, SC], BF16, tag="rlbc")
                    nc.scalar.copy(rlbc[:], psbc[:])
                    nc.vector.tensor_tensor(
                        outT[:, h, :],
                        pso[:],
                        rlbc[:],
                        op=mybir.AluOpType.mult,
                    )

            # ---- Phase C: y = outT^T @ wo (stream wo in n-chunks) ----
            with tc.tile_pool(name="phC", bufs=2) as pc, tc.tile_pool(
                name="phCy", bufs=4
            ) as pcy:
                y_tiles = [
                    pcy.tile([P, D], F32, tag="y", name=f"y{i}")
                    for i in range(NQTL)
                ]
                for ncl in range(4):
                    wo_g = pc.tile([P, NH, SC], BF16, tag="wog")
                    nc.sync.dma_start(wo_g[:], wo_d[:, ncl])
                    for qtl in range(NQTL):
                        psy = ps.tile([P, SC], F32, tag="big", name="psy")
                        psy = psy[:, :SC]
                        for h in range(NH):
                            nc.tensor.matmul(
                                psy,
                                outT[:, h, qtl * P : (qtl + 1) * P],
                                wo_g[:, h, :],
                                start=(h == 0),
                                stop=(h == NH - 1),
                            )
                        nc.vector.tensor_copy(
                            y_tiles[qtl][:, ncl * SC : (ncl + 1) * SC], psy
                        )
                for qtl in range(NQTL):
                    nc.sync.dma_start(
                        y_d[qtl * P : (qtl + 1) * P, :], y_tiles[qtl][:]
                    )

    return nc


_PROGRAM = None


def _get_program():
    global _PROGRAM
    if _PROGRAM is None:
        _PROGRAM = build_program()
        _PROGRAM.finalize()
    return _PROGRAM


def _host_inputs(x, wq, wk, wv, wo):
    """Per-core input packing (all arrays contiguous, uniform shapes)."""
    x2 = np.asarray(x, np.float32).reshape(S, D)
    xT = np.ascontiguousarray(x2.T)  # [D, S]
    xr = xT.reshape(KO, P, S)  # [ko, p, s]

    # paired RoPE basis permutation within each head
    perm = np.concatenate([np.arange(0, DQK, 2), np.arange(1, DQK, 2)])
    wq_p = np.asarray(wq, np.float32).reshape(D, NH, DQK)[:, :, perm]
    wk_p = np.asarray(wk, np.float32).reshape(D, NKV, DQK)[:, :, perm]
    wv_r = np.asarray(wv, np.float32).reshape(D, NKV * DV)
    wo_r = np.asarray(wo, np.float32).reshape(NH, DV, D)

    # device layouts independent of core
    wq_dev = np.ascontiguousarray(
        wq_p.reshape(KO, P, NH, DQK)  # [ko, p, h, dqk]
        .reshape(KO, P, 4, 4 * DQK)  # group 4 heads
        .transpose(1, 2, 0, 3)  # [p, g, ko, 4*dqk]
    ).astype(ml_bf16)
    wk_dev = np.ascontiguousarray(
        wk_p.reshape(KO, P, NKV * DQK).transpose(1, 0, 2)
    ).astype(ml_bf16)
    wv_dev = np.ascontiguousarray(
        wv_r.reshape(KO, P, NKV * DV).transpose(1, 0, 2)
    ).astype(ml_bf16)
    wo_dev = np.ascontiguousarray(
        wo_r.reshape(NH, DV, 4, SC).transpose(1, 2, 0, 3)  # [dv, ncl, h, sc]
    ).astype(ml_bf16)

    inv_freq = 1.0 / (THETA ** (np.arange(0, DQK, 2)[: DQK // 2] / DQK))

    in_maps = []
    for c in range(NC_):
        qlo = c * SC
        band_lo = qlo - WIN
        # packed key positions; garbage (pos<0) -> position 0, zero x
        pos = np.empty(NKC, np.int64)
        pos[: FRONT] = np.arange(FRONT)
        pos[FRONT:] = band_lo + np.arange(NKC - FRONT)
        valid = pos >= 0
        pos_c = np.where(valid, pos, 0)

        xp = xr[:, :, pos_c] * valid[None, None, :]  # [ko, p, nkc]
        xp = np.ascontiguousarray(xp.transpose(1, 0, 2)).astype(ml_bf16)

        ang = np.outer(pos_c.astype(np.float64), inv_freq)  # (nkc, 64)
        cos_h = np.cos(ang).T.astype(np.float32)  # (64, nkc)
        sin_h = np.sin(ang).T.astype(np.float32)
        cos_p = np.ascontiguousarray(np.concatenate([cos_h, cos_h], 0))
        sin_p = np.ascontiguousarray(np.concatenate([sin_h, sin_h], 0))

        # B mask [r, tile, q] -> [P, NT*SC]; front tiles use only the
        # front_ok condition, band tiles only the band condition (disjoint)
        r = np.arange(P)
        qpos = qlo + np.arange(SC)[None, :]  # [1, SC]
        B = np.zeros((P, NT, SC), np.float32)
        for t in range(NT):
            if t < 2:
                kpos = t * P + r[:, None]  # [P, 1]
                allowed = (kpos < FRONT) & (kpos <= qpos - WIN)
            else:
                kpos = band_lo + (t - 2) * P + r[:, None]
                allowed = (kpos >= 0) & (kpos <= qpos) & (kpos > qpos - WIN)
            B[:, t, :] = allowed
        Bp = np.ascontiguousarray(B.reshape(P, NT * SC)).astype(ml_bf16)

        in_maps.append(
            {
                "xp": xp,
                "wq": wq_dev,
                "wk": wk_dev,
                "wv": wv_dev,
                "wo": wo_dev,
                "cosd": cos_p,
                "sind": sin_p,
                "bmask": Bp,
            }
        )
    return in_maps


def kernel(x, wq, wk, wv, wo, _trace=False, _trace_kwargs=None):
    nc = _get_program()
    in_maps = _host_inputs(x, wq, wk, wv, wo)
    res = run_bass_kernel_spmd(
        nc, in_maps, list(range(NC_)), trace=_trace, **(_trace_kwargs or {})
    )
    y = np.concatenate(
        [np.asarray(r["y"], np.float32) for r in res.results], axis=0
    )
    out = y.reshape(1, S, D)
    if _trace:
        return out, res
    return out
